# revision 13
# baseline (speedup 1.0000x reference)
"""Trainium2 Bass kernel for nn_MixtralSparseMoeBlock_49486613184823.

Distributes the T=65536 token dimension across 8 NeuronCores. Per core:
  - transpose x tiles on the PE, matmul1 (f32r) -> patchesT, tanh
  - router logits for 3 gates, keys = sigmoid(|d|), masks = d>0
  - AllGather masked/shifted keys (tiny), replicated bisection search for
    per-expert top-fraction thresholds
  - matmul2 (bf16) -> feats2, selection-weighted reductions via PE
  - AllReduce partial sums (tiny), replicated final math
Outputs the full reference tuple; router_logits is token-sharded and
stitched on the host.
"""
import os
import sys

for _p in ("/opt/trn_rl_repo", "/root/.axon_site/_ro/trn_rl_repo"):
    if os.path.isdir(_p) and _p not in sys.path:
        sys.path.insert(0, _p)

import numpy as np

import concourse.bass as bass
import concourse.mybir as mybir
import concourse.tile as tile
from concourse import bacc
from concourse.bass_utils import run_bass_kernel_spmd
from concourse.masks import make_identity

F32 = mybir.dt.float32
F32R = mybir.dt.float32r
BF16 = mybir.dt.bfloat16
I32 = mybir.dt.int32
AF = mybir.ActivationFunctionType
OP = mybir.AluOpType

NCORES = 8
T = 65536
D = 1024
P = 512
TL = T // NCORES          # 8192 tokens per core
NHB = TL // 256           # 32 half-blocks of 256 tokens
NCH = TL // 128           # 64 chunks of 128 tokens
GNCH = T // 128           # 512 global chunks
FRACS = (0.25, 0.5, 0.25)
KEY_SHIFT = 0.4           # search operates on (key - KEY_SHIFT) * mask
SEARCH_LO = 0.0999
SEARCH_HI = 0.432
N_ITERS = 19

_CACHE = {}


def _build():
    nc = bacc.Bacc("TRN2", target_bir_lowering=False, debug=False,
                   num_devices=NCORES)

    x_d = nc.dram_tensor("x_shard", [TL, D], F32, kind="ExternalInput").ap()
    wt1_d = nc.dram_tensor("W_t1", [D, P], F32, kind="ExternalInput").ap()
    wcls1_d = nc.dram_tensor("W_cls1", [P, 2], F32, kind="ExternalInput").ap()
    wg_d = [nc.dram_tensor(f"Wg{g}", [P, 2], F32, kind="ExternalInput").ap()
            for g in range(3)]
    wclf_d = nc.dram_tensor("W_clf", [3, P, 3], F32, kind="ExternalInput").ap()
    wa1_d = nc.dram_tensor("W_a1", [P, P], F32, kind="ExternalInput").ap()
    wacls_d = nc.dram_tensor("W_acls", [P, 2], F32, kind="ExternalInput").ap()

    o_eyl = nc.dram_tensor("o_eyl", [1, 2], F32, kind="ExternalOutput").ap()
    o_eyh = nc.dram_tensor("o_eyh", [1, 1], I32, kind="ExternalOutput").ap()
    o_yl = nc.dram_tensor("o_yl", [1, 2], F32, kind="ExternalOutput").ap()
    o_yp = nc.dram_tensor("o_yp", [1, 2], F32, kind="ExternalOutput").ap()
    o_yh = nc.dram_tensor("o_yh", [1, 1], I32, kind="ExternalOutput").ap()
    o_rl = nc.dram_tensor("o_rl", [TL, 2], F32, kind="ExternalOutput").ap()
    o_jl = nc.dram_tensor("o_jl", [1, 1], F32, kind="ExternalOutput").ap()
    o_dist = nc.dram_tensor("o_dist", [1, 4], I32, kind="ExternalOutput").ap()
    o_dbg = nc.dram_tensor("o_dbg", [1, 24], F32, kind="ExternalOutput").ap()
    o_mk = nc.dram_tensor("o_mk", [128, 3 * NCH], F32, kind="ExternalOutput").ap()

    with tile.TileContext(nc) as tc:
        _emit(nc, tc, x_d, wt1_d, wcls1_d, wg_d, wclf_d, wa1_d, wacls_d,
              o_eyl, o_eyh, o_yl, o_yp, o_yh, o_rl, o_jl, o_dist, o_dbg, o_mk)
    nc.compile()
    return nc


def _emit(nc, tc, x_d, wt1_d, wcls1_d, wg_d, wclf_d, wa1_d, wacls_d,
          o_eyl, o_eyh, o_yl, o_yp, o_yh, o_rl, o_jl, o_dist, o_dbg, o_mk):
    from contextlib import ExitStack
    ctx = ExitStack()
    rg = [list(range(NCORES))]

    const = ctx.enter_context(tc.tile_pool(name="const", bufs=1))
    store = ctx.enter_context(tc.tile_pool(name="store", bufs=1))
    dram = ctx.enter_context(tc.tile_pool(name="dram", bufs=1, space="DRAM"))

    # identities
    ident = const.tile([128, 128], F32)
    make_identity(nc, ident)
    ident_fr = const.tile([128, 128], F32R)
    nc.vector.tensor_copy(ident_fr[:], ident[:])
    ident_bf = const.tile([128, 128], BF16)
    nc.vector.tensor_copy(ident_bf[:], ident[:])
    ones128 = const.tile([128, 128], F32)
    nc.vector.memset(ones128[:], 1.0)

    # ---- weights ----
    wt1_fr = const.tile([128, 8 * P], F32R)
    wa1_bf = const.tile([128, 4 * P], BF16)
    wg_fr = const.tile([128, 4 * 6], F32R)
    with tc.tile_pool(name="wstage", bufs=1) as wstage:
        wt1_st = wstage.tile([128, 8 * P], F32)
        nc.sync.dma_start(wt1_st.rearrange("p (dc m) -> p dc m", dc=8),
                          wt1_d.rearrange("(dc p) m -> p dc m", p=128))
        nc.vector.tensor_copy(wt1_fr[:], wt1_st[:])

        wa1_stage = wstage.tile([128, 4 * P], F32)
        nc.sync.dma_start(wa1_stage.rearrange("p (pc n) -> p pc n", pc=4),
                          wa1_d.rearrange("(pc p) n -> p pc n", p=128))
        nc.vector.tensor_copy(wa1_bf[:], wa1_stage[:])

        wg_stage = wstage.tile([128, 4 * 6], F32)
        for g in range(3):
            nc.sync.dma_start(
                wg_stage.rearrange("p (pc j) -> p pc j", pc=4)[:, :, 2 * g:2 * g + 2],
                wg_d[g].rearrange("(pc p) j -> p pc j", p=128))
        nc.vector.tensor_copy(wg_fr[:], wg_stage[:])

    wclf_sb = const.tile([128, 3 * 4 * 3], F32)
    nc.sync.dma_start(wclf_sb.rearrange("p (e pc c) -> p e pc c", e=3, pc=4),
                      wclf_d.rearrange("e (pc p) c -> p e pc c", p=128))
    wcls1_sb = const.tile([128, 4 * 2], F32)
    nc.sync.dma_start(wcls1_sb.rearrange("p (pc c) -> p pc c", pc=4),
                      wcls1_d.rearrange("(pc p) c -> p pc c", p=128))
    wacls_sb = const.tile([128, 4 * 2], F32)
    nc.sync.dma_start(wacls_sb.rearrange("p (pc c) -> p pc c", pc=4),
                      wacls_d.rearrange("(pc p) c -> p pc c", p=128))

    # ---- persistent stores ----
    patches = store.tile([128, NCH * P], BF16)      # [p, ch*512+f] token-major
    feats2 = store.tile([128, NCH * P], BF16)
    rt_store = store.tile([128, NCH * 6], F32)      # [p, ch*6 + (g*2+j)]
    mk3_loc = store.tile([128, 3 * NCH], F32)       # [p, g*64+ch]
    mk3 = store.tile([128, 3 * GNCH], F32)          # [p, g*512 + c*64+ch]
    w5 = store.tile([128, NCH * 5], BF16)           # [p, ch*5 + col]

    # =====================  PHASE 1  =====================
    with tc.tile_pool(name="xin", bufs=3) as xpool, \
         tc.tile_pool(name="xt", bufs=2) as xtpool, \
         tc.tile_pool(name="pt", bufs=2) as ptpool, \
         tc.tile_pool(name="rsb", bufs=2) as rpool, \
         tc.tile_pool(name="psA", bufs=2, space="PSUM") as psA, \
         tc.tile_pool(name="psM", bufs=2, space="PSUM") as psM, \
         tc.tile_pool(name="psR", bufs=1, space="PSUM") as psR, \
         tc.tile_pool(name="psB", bufs=2, space="PSUM") as psB:
        for hb in range(NHB):
            xts = []
            for tt in range(2):
                ch = hb * 2 + tt
                x_t = xpool.tile([128, D], F32, tag="x")
                nc.sync.dma_start(x_t[:], x_d[ch * 128:(ch + 1) * 128, :])
                xts.append(x_t)

            xT = xtpool.tile([128, 8 * 256], F32R, tag="xT")
            for dcp in range(4):
                pxT = psA.tile([128, 512], F32, tag="pxT")
                for dc2 in range(2):
                    dc = dcp * 2 + dc2
                    for tt in range(2):
                        nc.tensor.transpose(
                            pxT[:, dc2 * 256 + tt * 128:dc2 * 256 + tt * 128 + 128],
                            xts[tt][:, dc * 128:(dc + 1) * 128],
                            ident[:])
                nc.vector.tensor_copy(xT[:, dcp * 512:(dcp + 1) * 512], pxT[:])

            pT = ptpool.tile([128, 4 * 256], F32R, tag="pT")
            for ps in range(4):
                pm = psM.tile([128, 256], F32, tag="pm")
                for dc in range(8):
                    nc.tensor.matmul(
                        pm[:],
                        wt1_fr[:, dc * P + ps * 128:dc * P + ps * 128 + 128],
                        xT[:, dc * 256:(dc + 1) * 256],
                        start=(dc == 0), stop=(dc == 7))
                nc.scalar.activation(pT[:, ps * 256:(ps + 1) * 256], pm[:], AF.Tanh)

            # router logits for this half-block
            pr = psR.tile([6, 256], F32, tag="pr")
            for pc in range(4):
                nc.tensor.matmul(pr[:], wg_fr[:, pc * 6:(pc + 1) * 6],
                                 pT[:, pc * 256:(pc + 1) * 256],
                                 start=(pc == 0), stop=(pc == 3))
            rsb = rpool.tile([6, 256], F32, tag="rsb")
            nc.scalar.copy(rsb[:], pr[:])
            # gate0 logits -> output (transposing elementwise DMA)
            nc.sync.dma_start(
                o_rl[hb * 256:(hb + 1) * 256, :].rearrange("t j -> j t"),
                rsb[0:2, :])
            # transpose router rows -> token-major store
            prT = psR.tile([128, 12], F32, tag="prT")
            for tt in range(2):
                nc.tensor.transpose(prT[:, tt * 6:tt * 6 + 6],
                                    rsb[:, tt * 128:(tt + 1) * 128],
                                    ident[0:6, 0:6])
            nc.vector.tensor_copy(
                rt_store[:, (hb * 2) * 6:(hb * 2) * 6 + 12], prT[:])

            # patchesT -> token-major bf16 patches store
            for ps in range(4):
                pb = psB.tile([128, 256], F32R, tag="pb")
                for tt in range(2):
                    nc.tensor.transpose(
                        pb[:, tt * 128:(tt + 1) * 128],
                        pT[:, ps * 256 + tt * 128:ps * 256 + tt * 128 + 128],
                        ident_fr[:])
                nc.vector.tensor_copy(
                    patches.rearrange("p (ch f) -> p ch f", ch=NCH)
                    [:, hb * 2:hb * 2 + 2, ps * 128:ps * 128 + 128],
                    pb.rearrange("p (tt f) -> p tt f", tt=2))

    # =====================  PHASE 1b: keys/masks + AllGather  ============
    srch = ctx.enter_context(tc.tile_pool(name="srch", bufs=1))
    d3 = srch.tile([128, 3 * NCH], F32)
    rt3 = rt_store.rearrange("p (ch k) -> p ch k", ch=NCH)
    for g in range(3):
        dg = d3.rearrange("p (g ch) -> p g ch", g=3)[:, g, :]
        nc.vector.tensor_sub(dg, rt3[:, :, 2 * g + 1], rt3[:, :, 2 * g])
    # w_top0 = sigmoid(|d0|) is the sort key for ALL experts; only the
    # mask is per-gate.
    d0 = d3[:, 0:NCH]
    ad0 = srch.tile([128, NCH], F32)
    nc.vector.tensor_scalar(out=ad0[:], in0=d0, scalar1=-1.0, scalar2=None,
                            op0=OP.mult)
    nc.vector.tensor_max(ad0[:], ad0[:], d0)
    key0 = srch.tile([128, NCH], F32)
    nc.scalar.activation(key0[:], ad0[:], AF.Sigmoid)
    ms3 = srch.tile([128, 3 * NCH], F32)
    nc.vector.tensor_single_scalar(ms3[:], d3[:], 0.0, OP.is_gt)
    for g in range(3):
        nc.vector.scalar_tensor_tensor(
            out=mk3_loc[:, g * NCH:(g + 1) * NCH], in0=key0[:],
            scalar=KEY_SHIFT, in1=ms3[:, g * NCH:(g + 1) * NCH],
            op0=OP.subtract, op1=OP.mult)

    ag_in = dram.tile([128, 3 * NCH], F32)
    ag_out = dram.tile([128 * NCORES, 3 * NCH], F32, addr_space="Shared")
    nc.sync.dma_start(ag_in[:], mk3_loc[:])
    nc.gpsimd.collective_compute("AllGather", OP.bypass, replica_groups=rg,
                                 ins=[ag_in.opt()], outs=[ag_out.opt()])
    nc.sync.dma_start(
        mk3.rearrange("p (g c ch) -> p g c ch", g=3, c=NCORES),
        ag_out.rearrange("(c p) (g ch) -> p g c ch", p=128, g=3))

    # =====================  PHASE S: bisection search  ====================
    def rep3(name):
        t = srch.tile([128, 3], F32, name=name)
        return t

    cmp3 = srch.tile([128, 3 * GNCH], BF16)
    cw3 = rep3("cw3")
    cnt3 = rep3("cnt3")
    nums3 = rep3("nums3")
    lo3 = rep3("lo3")
    hi3 = rep3("hi3")
    tau3 = rep3("tau3")
    cnum3 = rep3("cnum3")
    pred3 = rep3("pred3")
    predn3 = rep3("predn3")
    chi3 = rep3("chi3")
    clo3 = rep3("clo3")
    frac3c = rep3("frac3c")
    numi3 = srch.tile([128, 3], I32)
    predi = srch.tile([128, 3], I32)
    predj = srch.tile([128, 3], I32)
    psS = ctx.enter_context(tc.tile_pool(name="psS", bufs=2, space="PSUM"))

    def gslice(t, g):
        return t[:, g * GNCH:(g + 1) * GNCH]

    def count_into(dst, thr):
        """dst[128,3] <- replicated global count of (mk3_g > thr_g)."""
        for g in range(3):
            nc.vector.tensor_scalar(
                out=gslice(cmp3, g), in0=gslice(mk3, g),
                scalar1=thr if isinstance(thr, float) else thr[:, g:g + 1],
                scalar2=0.0, op0=OP.is_gt, op1=OP.add,
                accum_out=cw3[:, g:g + 1])
        pc_ = psS.tile([128, 3], F32, tag="psS")
        nc.tensor.matmul(pc_[:], ones128[:], cw3[:], start=True, stop=True)
        nc.scalar.copy(dst[:], pc_[:])

    count_into(cnt3, 0.05)
    # nums = floor(cnt*frac), or cnt if floor==0
    nc.vector.memset(frac3c[:, 0:1], FRACS[0])
    nc.vector.memset(frac3c[:, 1:2], FRACS[1])
    nc.vector.memset(frac3c[:, 2:3], FRACS[2])
    nc.vector.tensor_mul(nums3[:], cnt3[:], frac3c[:])
    nc.vector.tensor_copy(numi3[:], nums3[:])
    nc.vector.tensor_copy(pred3[:], numi3[:])       # pred3 = round(cnt*frac)
    nc.vector.tensor_tensor(predn3[:], pred3[:], nums3[:], OP.is_gt)
    nc.vector.tensor_sub(nums3[:], pred3[:], predn3[:])   # exact floor
    nc.vector.tensor_single_scalar(predi[:], nums3[:], 0.5, OP.is_lt)
    nc.vector.copy_predicated(nums3[:], predi[:], cnt3[:])

    nc.vector.memset(lo3[:], SEARCH_LO)
    nc.vector.memset(hi3[:], SEARCH_HI)
    nc.vector.tensor_add(tau3[:], lo3[:], hi3[:])
    nc.vector.tensor_scalar(out=tau3[:], in0=tau3[:], scalar1=0.5,
                            scalar2=None, op0=OP.mult)
    for _ in range(N_ITERS):
        count_into(cnum3, tau3)
        nc.vector.tensor_tensor(predi[:], cnum3[:], nums3[:], OP.is_ge)
        nc.vector.copy_predicated(lo3[:], predi[:], tau3[:])
        nc.vector.tensor_tensor(predj[:], cnum3[:], nums3[:], OP.is_lt)
        nc.vector.copy_predicated(hi3[:], predj[:], tau3[:])
        nc.vector.tensor_add(tau3[:], lo3[:], hi3[:])
        nc.vector.tensor_scalar(out=tau3[:], in0=tau3[:], scalar1=0.5,
                                scalar2=None, op0=OP.mult)
    count_into(chi3, hi3)
    count_into(clo3, lo3)
    # frac = (nums - chi) / max(clo - chi, 1)
    q3 = rep3("q3")
    b3 = rep3("b3")
    nc.vector.tensor_sub(q3[:], nums3[:], chi3[:])
    nc.vector.tensor_sub(b3[:], clo3[:], chi3[:])
    nc.vector.tensor_scalar_max(b3[:], b3[:], 1.0)
    nc.vector.reciprocal(b3[:], b3[:])
    nc.vector.tensor_mul(frac3c[:], q3[:], b3[:])

    # local selection weights -> w5 (bf16): cols w0,w1,w2,ones,wtot
    wa = srch.tile([128, NCH], F32)
    wb = srch.tile([128, NCH], F32)
    wsum = srch.tile([128, NCH], F32)
    w5r = w5.rearrange("p (ch k) -> p ch k", ch=NCH)
    nc.vector.memset(wsum[:], 0.0)
    for g in range(3):
        mloc = mk3_loc[:, g * NCH:(g + 1) * NCH]
        nc.vector.tensor_scalar(out=wa[:], in0=mloc, scalar1=hi3[:, g:g + 1],
                                scalar2=None, op0=OP.is_gt)
        nc.vector.tensor_scalar(out=wb[:], in0=mloc, scalar1=lo3[:, g:g + 1],
                                scalar2=None, op0=OP.is_gt)
        nc.vector.tensor_sub(wb[:], wb[:], wa[:])
        nc.vector.tensor_scalar(out=wb[:], in0=wb[:],
                                scalar1=frac3c[:, g:g + 1], scalar2=None,
                                op0=OP.mult)
        nc.vector.tensor_add(wa[:], wa[:], wb[:])
        nc.vector.tensor_copy(w5r[:, :, g], wa[:])
        nc.vector.tensor_add(wsum[:], wsum[:], wa[:])
    nc.vector.memset(w5r[:, :, 3], 1.0)
    nc.vector.tensor_copy(w5r[:, :, 4], wsum[:])

    # =====================  PHASE D: mm2 (deferred, overlaps search)  =====
    with tc.tile_pool(name="pt2", bufs=2) as pt2pool, \
         tc.tile_pool(name="psD", bufs=2, space="PSUM") as psD, \
         tc.tile_pool(name="psD2", bufs=2, space="PSUM") as psD2:
        pch = patches.rearrange("p (ch f) -> p ch f", ch=NCH)
        fch = feats2.rearrange("p (ch f) -> p ch f", ch=NCH)
        for ch in range(NCH):
            pD = psD.tile([128, 512], BF16, tag="pD")
            for ps in range(4):
                nc.tensor.transpose(pD[:, ps * 128:(ps + 1) * 128],
                                    pch[:, ch, ps * 128:(ps + 1) * 128],
                                    ident_bf[:])
            pT2 = pt2pool.tile([128, 512], BF16, tag="pT2")
            nc.vector.tensor_copy(pT2[:], pD[:])
            pm2 = psD2.tile([128, 512], F32, tag="pm2")
            for ps in range(4):
                nc.tensor.matmul(pm2[:], pT2[:, ps * 128:(ps + 1) * 128],
                                 wa1_bf[:, ps * P:(ps + 1) * P],
                                 start=(ps == 0), stop=(ps == 3))
            nc.scalar.activation(fch[:, ch, :], pm2[:], AF.Tanh)

    # =====================  PHASE W: weighted reductions  =================
    psW = ctx.enter_context(tc.tile_pool(name="psW", bufs=1, space="PSUM"))
    pR = psW.tile([4, 512], F32, name="pR")
    pF = psW.tile([1, 512], F32, name="pF")
    pch = patches.rearrange("p (ch f) -> p ch f", ch=NCH)
    fch = feats2.rearrange("p (ch f) -> p ch f", ch=NCH)
    w5r2 = w5.rearrange("p (ch k) -> p ch k", ch=NCH)
    for ch in range(NCH):
        nc.tensor.matmul(pR[:], w5r2[:, ch, 0:4], pch[:, ch, :],
                         start=(ch == 0), stop=(ch == NCH - 1))
        nc.tensor.matmul(pF[:], w5r2[:, ch, 4:5], fch[:, ch, :],
                         start=(ch == 0), stop=(ch == NCH - 1))
    red_p = srch.tile([4, 512], F32)
    red_f = srch.tile([1, 512], F32)
    nc.scalar.copy(red_p[:], pR[:])
    nc.scalar.copy(red_f[:], pF[:])

    ar_in = dram.tile([5, 512], F32)
    ar_out = dram.tile([5, 512], F32, addr_space="Shared")
    nc.sync.dma_start(ar_in[0:4, :], red_p[:])
    nc.sync.dma_start(ar_in[4:5, :], red_f[:])
    nc.gpsimd.collective_compute("AllReduce", OP.add, replica_groups=rg,
                                 ins=[ar_in.opt()], outs=[ar_out.opt()])
    red_g1 = srch.tile([1, 5 * 512], F32)
    for r in range(5):
        nc.sync.dma_start(red_g1[0:1, r * 512:(r + 1) * 512], ar_out[r:r + 1, :])

    # =====================  PHASE F: final math  ==========================
    fin = srch
    psF = ctx.enter_context(tc.tile_pool(name="psF", bufs=1, space="PSUM"))

    rnum = rep3("rnum")
    nc.vector.tensor_scalar_max(rnum[:], nums3[:], 1.0)
    nc.vector.reciprocal(rnum[:], rnum[:])
    tot_n = fin.tile([1, 1], F32)
    nc.vector.tensor_reduce(tot_n[:], nums3[0:1, :], mybir.AxisListType.X,
                            OP.add)
    nc.vector.tensor_scalar_max(tot_n[:], tot_n[:], 1.0)
    nc.vector.reciprocal(tot_n[:], tot_n[:])

    vecs = fin.tile([1, 4 * 512], F32)   # segs: mf0, mf1, agg_bag, bag
    for e in range(2):
        nc.vector.tensor_scalar(out=vecs[0:1, e * 512:(e + 1) * 512],
                                in0=red_g1[0:1, e * 512:(e + 1) * 512],
                                scalar1=rnum[0:1, e:e + 1], scalar2=None,
                                op0=OP.mult)
    nc.vector.tensor_scalar(out=vecs[0:1, 2 * 512:3 * 512],
                            in0=red_g1[0:1, 4 * 512:5 * 512],
                            scalar1=tot_n[0:1, 0:1], scalar2=None, op0=OP.mult)
    nc.vector.tensor_scalar(out=vecs[0:1, 3 * 512:4 * 512],
                            in0=red_g1[0:1, 3 * 512:4 * 512],
                            scalar1=1.0 / T, scalar2=None, op0=OP.mult)

    pV = psF.tile([128, 16], F32, name="pV")
    for v in range(4):
        for pc in range(4):
            nc.tensor.transpose(pV[:, v * 4 + pc:v * 4 + pc + 1],
                                vecs[0:1, v * 512 + pc * 128:v * 512 + (pc + 1) * 128],
                                ident[0:1, 0:1])
    vT = fin.tile([128, 16], F32)
    nc.scalar.copy(vT[:], pV[:])

    le2 = fin.tile([1, 6], F32)      # logits for experts 0,1
    for e in range(2):
        pl = psF.tile([1, 3], F32, tag="pl")
        for pc in range(4):
            nc.tensor.matmul(pl[:], vT[:, e * 4 + pc:e * 4 + pc + 1],
                             wclf_sb[:, e * 12 + pc * 3:e * 12 + pc * 3 + 3],
                             start=(pc == 0), stop=(pc == 3))
        nc.scalar.copy(le2[0:1, e * 3:e * 3 + 3], pl[:])
    ey = fin.tile([1, 2], F32)
    pl2 = psF.tile([1, 2], F32, name="pl2")
    for pc in range(4):
        nc.tensor.matmul(pl2[:], vT[:, 2 * 4 + pc:2 * 4 + pc + 1],
                         wacls_sb[:, pc * 2:pc * 2 + 2],
                         start=(pc == 0), stop=(pc == 3))
    nc.scalar.copy(ey[:], pl2[:])
    yl = fin.tile([1, 2], F32)
    pl3 = psF.tile([1, 2], F32, name="pl3")
    for pc in range(4):
        nc.tensor.matmul(pl3[:], vT[:, 3 * 4 + pc:3 * 4 + pc + 1],
                         wcls1_sb[:, pc * 2:pc * 2 + 2],
                         start=(pc == 0), stop=(pc == 3))
    nc.scalar.copy(yl[:], pl3[:])

    # joint loss = sum_e<2 -log_softmax(le[e])[e]
    sc1 = fin.tile([1, 1], F32)
    sc2 = fin.tile([1, 1], F32)
    sc3 = fin.tile([1, 1], F32)
    ex3 = fin.tile([1, 3], F32)
    jl = fin.tile([1, 1], F32)
    nc.vector.memset(jl[:], 0.0)
    for e in range(2):
        lee = le2[0:1, e * 3:e * 3 + 3]
        nc.vector.tensor_reduce(sc1[:], lee, mybir.AxisListType.X, OP.max)
        nc.vector.tensor_scalar(out=sc2[:], in0=sc1[:], scalar1=-1.0,
                                scalar2=None, op0=OP.mult)
        nc.scalar.activation(ex3[:], lee, AF.Exp, bias=sc2[0:1, 0:1])
        nc.vector.tensor_reduce(sc3[:], ex3[:], mybir.AxisListType.X, OP.add)
        nc.scalar.activation(sc3[:], sc3[:], AF.Ln)
        nc.vector.tensor_add(sc3[:], sc3[:], sc1[:])
        nc.vector.tensor_sub(sc3[:], sc3[:], le2[0:1, e * 3 + e:e * 3 + e + 1])
        nc.vector.tensor_add(jl[:], jl[:], sc3[:])
    nc.sync.dma_start(o_jl, jl[:])

    # Y_prob = softmax(yl)
    nc.vector.tensor_reduce(sc1[:], yl[:], mybir.AxisListType.X, OP.max)
    nc.vector.tensor_scalar(out=sc2[:], in0=sc1[:], scalar1=-1.0,
                            scalar2=None, op0=OP.mult)
    ex2 = fin.tile([1, 2], F32)
    nc.scalar.activation(ex2[:], yl[:], AF.Exp, bias=sc2[0:1, 0:1])
    nc.vector.tensor_reduce(sc3[:], ex2[:], mybir.AxisListType.X, OP.add)
    nc.vector.reciprocal(sc3[:], sc3[:])
    yp = fin.tile([1, 2], F32)
    nc.vector.tensor_scalar(out=yp[:], in0=ex2[:], scalar1=sc3[0:1, 0:1],
                            scalar2=None, op0=OP.mult)
    nc.sync.dma_start(o_yp, yp[:])
    nc.sync.dma_start(o_yl, yl[:])
    nc.sync.dma_start(o_eyl, ey[:])

    # argmaxes
    hatf = fin.tile([1, 1], F32)
    hati = fin.tile([1, 1], I32)
    nc.vector.tensor_tensor(hatf[:], yl[0:1, 1:2], yl[0:1, 0:1], OP.is_gt)
    nc.vector.tensor_copy(hati[:], hatf[:])
    nc.sync.dma_start(o_yh, hati[:])
    hatf2 = fin.tile([1, 1], F32)
    hati2 = fin.tile([1, 1], I32)
    nc.vector.tensor_tensor(hatf2[:], ey[0:1, 1:2], ey[0:1, 0:1], OP.is_gt)
    nc.vector.tensor_copy(hati2[:], hatf2[:])
    nc.sync.dma_start(o_eyh, hati2[:])

    # distribute
    distf = fin.tile([1, 4], F32)
    disti = fin.tile([1, 4], I32)
    nc.vector.memset(distf[0:1, 0:1], float(T))
    nc.vector.tensor_copy(distf[0:1, 1:4], nums3[0:1, :])
    nc.vector.tensor_copy(disti[:], distf[:])
    nc.sync.dma_start(o_dist, disti[:])

    dbg = srch.tile([1, 24], F32)
    for i, tsrc in enumerate((cnt3, nums3, chi3, clo3, lo3, hi3, tau3, frac3c)):
        nc.vector.tensor_copy(dbg[0:1, i * 3:(i + 1) * 3], tsrc[0:1, :])
    nc.sync.dma_start(o_dbg, dbg[:])
    nc.sync.dma_start(o_mk, mk3_loc[:])

    ctx.close()


def kernel(**inputs):
    if "nc" not in _CACHE:
        _CACHE["nc"] = _build()
    nc = _CACHE["nc"]

    x = np.ascontiguousarray(
        np.asarray(inputs["hidden_states"], np.float32).reshape(T, D))
    common = {
        "W_t1": np.ascontiguousarray(np.asarray(inputs["W_t1"], np.float32)),
        "W_cls1": np.ascontiguousarray(np.asarray(inputs["W_cls1"], np.float32)),
        "Wg0": np.ascontiguousarray(np.asarray(inputs["Wg0"], np.float32)),
        "Wg1": np.ascontiguousarray(np.asarray(inputs["Wg1"], np.float32)),
        "Wg2": np.ascontiguousarray(np.asarray(inputs["Wg2"], np.float32)),
        "W_clf": np.ascontiguousarray(np.asarray(inputs["W_clf"], np.float32)),
        "W_a1": np.ascontiguousarray(np.asarray(inputs["W_a1"], np.float32)),
        "W_acls": np.ascontiguousarray(np.asarray(inputs["W_acls"], np.float32)),
    }
    in_maps = []
    for c in range(NCORES):
        m = dict(common)
        m["x_shard"] = np.ascontiguousarray(x[c * TL:(c + 1) * TL])
        in_maps.append(m)

    res = run_bass_kernel_spmd(nc, in_maps, core_ids=list(range(NCORES)))
    _CACHE["last_result"] = res
    r0 = res.results[0]

    router_logits = np.concatenate(
        [res.results[c]["o_rl"] for c in range(NCORES)], axis=0)
    return (
        r0["o_eyl"].astype(np.float32),
        r0["o_eyh"].reshape(1).astype(np.int32),
        r0["o_yl"].astype(np.float32),
        r0["o_yp"].astype(np.float32),
        r0["o_yh"].reshape(1).astype(np.int32),
        router_logits.astype(np.float32),
        np.float32(r0["o_jl"].reshape(())),
        r0["o_dist"].reshape(4).astype(np.int32),
    )
